# revision 1
# baseline (speedup 1.0000x reference)
"""Trainium2 Bass kernel for nn_CustomABlock (MDTA transformer block).

Per-core layout: one batch image [C=256, N=4096(=64x64)] per NeuronCore,
data-parallel over B=8 across 8 cores, all params replicated.

Engine plan (per core):
  PE   : qkv matmul (f32r), 2 dwconv taps (diag matmul), q/k transposes,
         gram (attn logits), attn@v, proj, mlp1, mlp2
  DVE  : 6 dwconv taps (scalar_tensor_tensor FMA, bf16), residual adds,
         reciprocals, row-max reduces, x1 bf16 copy
  ACT  : PSUM drains, l2norm squares (accum), exp (softmax), gelu+bias
  GPSIMD: 1 dwconv tap, identity build
"""

import numpy as np
import ml_dtypes

BF16 = ml_dtypes.bfloat16

C = 256          # dim
N = 4096         # 64*64
H = W = 64
NH = 8           # heads
CH = 32          # channels per head
HID = 307        # mlp hidden
NB_QKV = 6       # qkv channel blocks of 128
NT = 8           # n tiles of 512
TS = 512

# tap index t = (dy+1)*3 + (dx+1)
PE_TAPS = [(0, 0), (-1, 0), (1, 0), (0, -1), (0, 1)]  # PE diag matmuls into PSUM
MERGE_TAP = (1, 1)                  # DVE STT: tap + PSUM drain in one op
DVE_TAPS = [(-1, -1), (-1, 1), (1, -1)]   # DVE tensor_scalar + tensor_tensor

_CACHE = {}


def _build_bass():
    import concourse.bass as bass
    from concourse import bacc
    from concourse import mybir
    from concourse.tile import TileContext
    from concourse.masks import make_identity

    dt = mybir.dt
    f32 = dt.float32
    f32r = dt.float32r
    bf16 = dt.bfloat16
    AF = mybir.ActivationFunctionType
    OP = mybir.AluOpType

    nc = bacc.Bacc("TRN2")

    # ---- DRAM I/O (per-core) ----
    x_d = nc.dram_tensor("x", [128, 2, N], f32, kind="ExternalInput")
    xb_d = nc.dram_tensor("xb", [128, 2, N], bf16, kind="ExternalInput")
    wqkv_d = nc.dram_tensor("wqkvT", [128, 2, 3 * C], bf16, kind="ExternalInput")
    wdiag_d = nc.dram_tensor("wdiag", [128, len(PE_TAPS), NB_QKV, 128], bf16,
                             kind="ExternalInput")
    wdw_d = nc.dram_tensor("wdw", [128, NB_QKV * 9 * 2], f32, kind="ExternalInput")
    wproj_d = nc.dram_tensor("wprojT", [128, 2, C], bf16, kind="ExternalInput")
    wm1_d = nc.dram_tensor("wm1T", [128, 2, HID], bf16, kind="ExternalInput")
    wm2_d = nc.dram_tensor("wm2T", [128, 3, C], bf16, kind="ExternalInput")
    b1_d = nc.dram_tensor("b1", [128, 3], f32, kind="ExternalInput")
    b2_d = nc.dram_tensor("b2", [128, 2], f32, kind="ExternalInput")
    tv_d = nc.dram_tensor("tempvec", [128, 2], f32, kind="ExternalInput")
    out_d = nc.dram_tensor("out", [128, 2, N], f32, kind="ExternalOutput")

    with TileContext(nc) as tc:
        with (
            tc.tile_pool(name="wpool", bufs=1) as wpool,
            tc.tile_pool(name="xpool", bufs=1) as xpool,
            tc.tile_pool(name="qkvp", bufs=3) as qkvp,       # qkv_s blocks / ys reuse
            tc.tile_pool(name="dwqk", bufs=3) as dwqk_p,     # dw q/k blocks / attn_s reuse
            tc.tile_pool(name="dwv", bufs=2) as dwv_p,       # dw v blocks / x1b reuse
            tc.tile_pool(name="qt", bufs=1) as qt_p,
            tc.tile_pool(name="scr", bufs=2) as scr_p,
            tc.tile_pool(name="small", bufs=10) as small_p,
            tc.tile_pool(name="apool", bufs=2) as a_p,
            tc.tile_pool(name="pbig", bufs=2, space="PSUM") as pbig,
            tc.tile_pool(name="pdw", bufs=2, space="PSUM") as pdw,
            tc.tile_pool(name="psml", bufs=2, space="PSUM") as psml,
        ):
            # ---- load weights & x (critical path first) ----
            xs = xpool.tile([128, 2, N], f32)
            xr = xpool.tile([128, 2, N], bf16)
            wqkv_s = wpool.tile([128, 2, 3 * C], bf16)
            for kb in range(2):
                nc.sync.dma_start(out=wqkv_s[:, kb, :], in_=wqkv_d[:, kb, :])
                nc.sync.dma_start(out=xr[:, kb, :], in_=xb_d[:, kb, :])
            wdiag_s = wpool.tile([128, len(PE_TAPS), NB_QKV, 128], bf16)
            nc.sync.dma_start(out=wdiag_s, in_=wdiag_d[:, :, :, :])
            wdw_s = wpool.tile([128, NB_QKV * 9 * 2], f32)
            nc.sync.dma_start(out=wdw_s, in_=wdw_d[:, :])
            for kb in range(2):
                nc.sync.dma_start(out=xs[:, kb, :], in_=x_d[:, kb, :])
            wproj_s = wpool.tile([128, 2, C], bf16)
            nc.sync.dma_start(out=wproj_s, in_=wproj_d[:, :, :])
            wm1_s = wpool.tile([128, 2, HID], bf16)
            nc.sync.dma_start(out=wm1_s, in_=wm1_d[:, :, :])
            wm2_s = wpool.tile([128, 3, C], bf16)
            nc.sync.dma_start(out=wm2_s, in_=wm2_d[:, :, :])
            b1_s = wpool.tile([128, 3], f32)
            nc.sync.dma_start(out=b1_s, in_=b1_d[:, :])
            b2_s = wpool.tile([128, 2], f32)
            nc.sync.dma_start(out=b2_s, in_=b2_d[:, :])
            tv_s = wpool.tile([128, 2], f32)
            nc.sync.dma_start(out=tv_s, in_=tv_d[:, :])

            ident = wpool.tile([128, 128], bf16)
            make_identity(nc, ident)

            # ---- per-block pipeline ----
            dw_tiles = [None] * NB_QKV
            qT_s = qt_p.tile([128, 32, C], bf16, tag="qT")
            kT_s = qt_p.tile([128, 32, C], bf16, tag="kT")
            attn_s = [dwqk_p.tile([128, N], bf16, tag="dwqk", name=f"attn{g}")
                      for g in range(2)]
            rs_v = [None, None]
            At_v = [None, None]

            def do_block(ob):
                # qkv = W_qkv @ x  (bf16) -> PSUM [128,1024] -> bf16 SBUF
                qkv_t = qkvp.tile([128, N], bf16, tag="qkv", name=f"qkv{ob}")
                for t in range(4):
                    ps = pbig.tile([128, 1024], f32, tag="pbig", name="ps")
                    for h in range(2):
                        for kb in range(2):
                            nc.tensor.matmul(
                                ps[:, h * TS:(h + 1) * TS],
                                lhsT=wqkv_s[:, kb, ob * 128:(ob + 1) * 128],
                                rhs=xr[:, kb, t * 1024 + h * TS:
                                       t * 1024 + (h + 1) * TS],
                                start=(kb == 0), stop=(kb == 1),
                            )
                    nc.scalar.copy(out=qkv_t[:, t * 1024:(t + 1) * 1024], in_=ps)

                # dwconv: 5 PE diag taps (flat shifts) -> PSUM[128,512]
                dw_t = (dwqk_p if ob < 4 else dwv_p).tile(
                    [128, N], bf16, tag=("dwqk" if ob < 4 else "dwv"),
                    name=f"dw{ob}")
                dw_tiles[ob] = dw_t
                dw3 = dw_t.rearrange("p (y x) -> p y x", y=H)
                qk3 = qkv_t.rearrange("p (y x) -> p y x", y=H)
                dym, dxm = MERGE_TAP
                wm = wdw_s[:, ob * 9 + (dym + 1) * 3 + dxm + 1:
                           ob * 9 + (dym + 1) * 3 + dxm + 2]
                w01 = wdw_s[:, 54 + ob * 9 + 5:54 + ob * 9 + 6]
                for t8 in range(8):
                    pd = pdw.tile([128, TS], f32, tag="pdw", name="pd")
                    pd3 = pd.rearrange("p (y x) -> p y x", y=8)
                    c0 = t8 * TS
                    ops = []
                    for ti, (dy, dx) in enumerate(PE_TAPS):
                        s = dy * 64 + dx
                        a = max(c0, -s)
                        b = min(c0 + TS, N - max(0, s))
                        if a < b:
                            ops.append((ti, s, a, b))
                    for j, (ti, s, a, b) in enumerate(ops):
                        nc.tensor.matmul(
                            pd[:, a - c0:b - c0],
                            lhsT=wdiag_s[:, ti, ob, :],
                            rhs=qkv_t[:, a + s:b + s],
                            start=(j == 0), stop=(j == len(ops) - 1),
                        )
                    yt = t8 * 8
                    # merge tap (1,1): dw = w*qkv[y+1,x+1] + psum (drains)
                    ya, yb = yt, min(yt + 8, 63)
                    nc.vector.scalar_tensor_tensor(
                        out=dw3[:, ya:yb, 0:63],
                        in0=qk3[:, ya + 1:yb + 1, 1:64],
                        scalar=wm,
                        in1=pd3[:, 0:yb - yt, 0:63],
                        op0=OP.mult, op1=OP.add,
                    )
                    # x=63 col: drain PSUM minus tap(0,1) row-wrap
                    nc.vector.scalar_tensor_tensor(
                        out=dw3[:, yt:yb, 63:64],
                        in0=qk3[:, yt + 1:yb + 1, 0:1],
                        scalar=w01, in1=pd3[:, 0:yb - yt, 63:64],
                        op0=OP.mult, op1=OP.add,
                    )
                    if t8 == 7:
                        nc.scalar.copy(out=dw3[:, 63:64, :],
                                       in_=pd3[:, 7:8, :])
                # x=0 col: subtract tap(0,-1) row-wrap (whole block, in place)
                w0m = wdw_s[:, 54 + ob * 9 + 3:54 + ob * 9 + 4]
                nc.vector.scalar_tensor_tensor(
                    out=dw3[:, 1:64, 0:1], in0=qk3[:, 0:63, 63:64],
                    scalar=w0m, in1=dw3[:, 1:64, 0:1],
                    op0=OP.mult, op1=OP.add,
                )

                # 3 corner taps: tensor_scalar (4x) + tensor_tensor (2x)
                for (dy, dx) in DVE_TAPS:
                    ti = (dy + 1) * 3 + (dx + 1)
                    w_ap = wdw_s[:, ob * 9 + ti:ob * 9 + ti + 1]
                    y0, y1 = max(0, -dy), 64 - max(0, dy)
                    x0, x1 = max(0, -dx), 64 - max(0, dx)
                    sc_t = scr_p.tile([128, N], bf16, tag="sqscr",
                                      name=f"scr{ob}_{ti}")
                    sc3 = sc_t.rearrange("p (y x) -> p y x", y=H)
                    nc.vector.tensor_scalar_mul(
                        sc3[:, y0:y1, x0:x1],
                        qk3[:, y0 + dy:y1 + dy, x0 + dx:x1 + dx], w_ap)
                    nc.vector.tensor_tensor(
                        out=dw3[:, y0:y1, x0:x1], in0=dw3[:, y0:y1, x0:x1],
                        in1=sc3[:, y0:y1, x0:x1], op=OP.add)

                # q/k blocks: l2 norm row-scale then transpose to [n, c]
                if ob < 4:
                    sq = scr_p.tile([128, N], bf16, tag="sqscr")
                    ssq = small_p.tile([128, 1], f32, tag="ssq")
                    nc.scalar.activation(out=sq, in_=dw_t, func=AF.Square,
                                         accum_out=ssq)
                    nrm = small_p.tile([128, 1], f32, tag="nrm")
                    nc.scalar.sqrt(nrm, ssq)
                    rn = small_p.tile([128, 1], f32, tag="rn")
                    nc.vector.reciprocal(rn, nrm)
                    if ob < 2:   # q rows: fold temperature in
                        sc = small_p.tile([128, 1], f32, tag="sc")
                        nc.vector.tensor_mul(sc, rn, tv_s[:, ob:ob + 1])
                        rowscale = sc
                    else:
                        rowscale = rn
                    nc.vector.tensor_scalar_mul(dw_t, dw_t, rowscale)
                    dst = qT_s if ob < 2 else kT_s
                    cof = (ob % 2) * 128
                    for g in range(8):
                        tp_t = psml.tile([128, 512], bf16, tag="tp")
                        for i in range(4):
                            nb = g * 4 + i
                            nc.tensor.transpose(
                                tp_t[:, i * 128:(i + 1) * 128],
                                dw_t[:, nb * 128:(nb + 1) * 128], ident)
                        nc.scalar.copy(
                            out=dst[:, g * 4:g * 4 + 4, cof:cof + 128],
                            in_=tp_t.rearrange("p (a b) -> p a b", a=4))

            def do_gram(g):
                pg = psml.tile([128, 128], f32, tag="tp")
                co = g * 128
                for nb in range(32):
                    nc.tensor.matmul(
                        pg,
                        lhsT=qT_s[:, nb, co:co + 128],
                        rhs=kT_s[:, nb, co:co + 128],
                        start=(nb == 0), stop=(nb == 31),
                    )
                A_t = a_p.tile([128, 128], bf16, tag="A")
                nc.vector.memset(A_t, 0.0)
                mx = small_p.tile([128, 1], f32, tag="mx")
                sm = small_p.tile([128, 1], f32, tag="sm")
                for h in range(4):
                    r0, r1 = h * 32, h * 32 + 32
                    nc.vector.tensor_reduce(
                        out=mx[r0:r1, :], in_=pg[r0:r1, r0:r1],
                        axis=mybir.AxisListType.X, op=OP.max)
                nc.vector.tensor_scalar_mul(mx, mx, -1.0)
                for h in range(4):
                    r0, r1 = h * 32, h * 32 + 32
                    nc.scalar.activation(
                        out=A_t[r0:r1, r0:r1], in_=pg[r0:r1, r0:r1],
                        func=AF.Exp, bias=mx[r0:r1, :],
                        accum_out=sm[r0:r1, :])
                rs = small_p.tile([128, 1], f32, tag="rs")
                nc.vector.reciprocal(rs, sm)
                rs_v[g] = rs
                pa = psml.tile([128, 128], bf16, tag="tp")
                nc.tensor.transpose(pa, A_t, ident)
                At = a_p.tile([128, 128], bf16, tag="At")
                nc.scalar.copy(out=At, in_=pa)
                At_v[g] = At

            def do_av(g):
                for t in range(4):
                    pv = pbig.tile([128, 1024], f32, tag="pbig", name="pv")
                    for h in range(2):
                        nc.tensor.matmul(
                            pv[:, h * TS:(h + 1) * TS], lhsT=At_v[g],
                            rhs=dw_tiles[4 + g][:, t * 1024 + h * TS:
                                                t * 1024 + (h + 1) * TS],
                            start=True, stop=True)
                    nc.scalar.mul(attn_s[g][:, t * 1024:(t + 1) * 1024],
                                  pv, rs_v[g])

            do_block(0)
            do_block(2)
            do_gram(0)
            do_block(1)
            do_block(3)
            do_gram(1)
            do_block(4)
            do_av(0)
            do_block(5)
            do_av(1)

            # ---- streamed tail: proj+resid1 / mlp1 / mlp2+resid2+DMA per tile ----
            x1b = [dwv_p.tile([128, N], bf16, tag="dwv", name=f"x1b{i}")
                   for i in range(2)]
            ys = [qkvp.tile([128, N], bf16, tag="qkv", name=f"ys{i}")
                  for i in range(3)]
            for t in range(4):
                sl = slice(t * 1024, (t + 1) * 1024)
                for ob in range(2):
                    pp = pbig.tile([128, 1024], f32, tag="pbig", name="pp")
                    for h in range(2):
                        for kb in range(2):
                            nc.tensor.matmul(
                                pp[:, h * TS:(h + 1) * TS],
                                lhsT=wproj_s[:, kb, ob * 128:(ob + 1) * 128],
                                rhs=attn_s[kb][:, t * 1024 + h * TS:
                                               t * 1024 + (h + 1) * TS],
                                start=(kb == 0), stop=(kb == 1))
                    nc.vector.tensor_tensor(
                        out=xs[:, ob, sl], in0=xs[:, ob, sl], in1=pp, op=OP.add)
                    nc.vector.tensor_copy(out=x1b[ob][:, sl], in_=xs[:, ob, sl])
                for mb in range(3):
                    rows = 128 if mb < 2 else HID - 256
                    pm = pbig.tile([128, 1024], f32, tag="pbig", name="pm")
                    for h in range(2):
                        for kb in range(2):
                            nc.tensor.matmul(
                                pm[:rows, h * TS:(h + 1) * TS],
                                lhsT=wm1_s[:, kb, mb * 128:mb * 128 + rows],
                                rhs=x1b[kb][:, t * 1024 + h * TS:
                                            t * 1024 + (h + 1) * TS],
                                start=(kb == 0), stop=(kb == 1))
                    nc.scalar.activation(
                        out=ys[mb][:rows, sl],
                        in_=pm[:rows, :], func=AF.Gelu_apprx_tanh,
                        bias=b1_s[:rows, mb:mb + 1])
                for ob in range(2):
                    pm2 = pbig.tile([128, 1024], f32, tag="pbig", name="pm2")
                    for h in range(2):
                        for kb in range(3):
                            rows = 128 if kb < 2 else HID - 256
                            nc.tensor.matmul(
                                pm2[:, h * TS:(h + 1) * TS],
                                lhsT=wm2_s[:rows, kb, ob * 128:(ob + 1) * 128],
                                rhs=ys[kb][:rows, t * 1024 + h * TS:
                                           t * 1024 + (h + 1) * TS],
                                start=(kb == 0), stop=(kb == 2))
                    nc.vector.scalar_tensor_tensor(
                        out=xs[:, ob, sl], in0=pm2,
                        scalar=b2_s[:, ob:ob + 1], in1=xs[:, ob, sl],
                        op0=OP.add, op1=OP.add)
                    nc.sync.dma_start(out=out_d[:, ob, sl], in_=xs[:, ob, sl])

    return nc


def _prep_shared(w_qkv, w_dw, temperature, w_proj, w_mlp1, b_mlp1, w_mlp2, b_mlp2):
    f32 = np.float32
    shared = {}
    shared["wqkvT"] = np.ascontiguousarray(
        w_qkv.T.reshape(2, 128, 3 * C).transpose(1, 0, 2)).astype(BF16)
    wd = np.zeros((128, len(PE_TAPS), NB_QKV, 128), BF16)
    for ti, (dy, dx) in enumerate(PE_TAPS):
        for cb in range(NB_QKV):
            w = w_dw[cb * 128:(cb + 1) * 128, 0, dy + 1, dx + 1].astype(f32)
            wd[:, ti, cb, :] = np.diag(w).astype(BF16)
    shared["wdiag"] = wd
    wt = np.zeros((128, NB_QKV * 9 * 2), f32)
    for cb in range(NB_QKV):
        for t in range(9):
            wt[:, cb * 9 + t] = w_dw[cb * 128:(cb + 1) * 128, 0, t // 3, t % 3]
    wt[:, 54:] = -wt[:, :54]
    shared["wdw"] = wt
    shared["wprojT"] = np.ascontiguousarray(
        w_proj.T.reshape(2, 128, C).transpose(1, 0, 2)).astype(BF16)
    shared["wm1T"] = np.ascontiguousarray(
        w_mlp1.T.reshape(2, 128, HID).transpose(1, 0, 2)).astype(BF16)
    w2 = np.zeros((384, C), f32)
    w2[:HID] = w_mlp2.T
    shared["wm2T"] = np.ascontiguousarray(
        w2.reshape(3, 128, C).transpose(1, 0, 2)).astype(BF16)
    b1 = np.zeros((384,), f32)
    b1[:HID] = b_mlp1
    shared["b1"] = np.ascontiguousarray(b1.reshape(3, 128).T)
    shared["b2"] = np.ascontiguousarray(b_mlp2.astype(f32).reshape(2, 128).T)
    t = temperature.reshape(NH).astype(f32)
    tv = np.zeros((128, 2), f32)
    for g in range(2):
        tv[:, g] = np.repeat(t[g * 4:(g + 1) * 4], 32)
    shared["tempvec"] = tv
    return shared


def kernel(x, w_qkv, w_dw, temperature, w_proj, w_mlp1, b_mlp1, w_mlp2, b_mlp2,
           _trace=False):
    from concourse.bass_utils import run_bass_kernel_spmd

    if "nc" not in _CACHE:
        nc = _build_bass()
        nc.finalize()
        _CACHE["nc"] = nc
    nc = _CACHE["nc"]

    x = np.asarray(x, np.float32)
    B = x.shape[0]
    shared = _prep_shared(
        np.asarray(w_qkv, np.float32), np.asarray(w_dw, np.float32),
        np.asarray(temperature, np.float32), np.asarray(w_proj, np.float32),
        np.asarray(w_mlp1, np.float32), np.asarray(b_mlp1, np.float32),
        np.asarray(w_mlp2, np.float32), np.asarray(b_mlp2, np.float32))

    in_maps = []
    for i in range(B):
        m = dict(shared)
        xi = np.ascontiguousarray(x[i].reshape(2, 128, N).transpose(1, 0, 2))
        m["x"] = xi
        m["xb"] = xi.astype(BF16)
        in_maps.append(m)

    res = run_bass_kernel_spmd(nc, in_maps, core_ids=list(range(B)),
                               trace=_trace)
    outs = np.stack([
        r["out"].transpose(1, 0, 2).reshape(C, H, W) for r in res.results
    ])
    if _trace:
        _CACHE["last_exec_ns"] = res.exec_time_ns
        _CACHE["last_profile"] = res.profile_json
    return outs

